# revision 1
# baseline (speedup 1.0000x reference)
"""HGAT-90 kernel: 4-layer GATv2 GNN, head-sharded across 8 trn2 NeuronCores.

Sharding strategy (per spec hint, adapted): weights replicated per-head-slice,
graph replicated; each of the 8 cores owns one attention head (128 of the 1024
hidden channels). Per layer each core computes its head's projections, the
per-edge segment softmax and scatter (fully local per head), partial LayerNorm
statistics (psum across cores), applies LN + residual to its slice, and
all-gathers slices into the full hidden state for the next layer's matmul.
The pooled classifier heads are computed as per-core partial products and
summed with a psum; the tiny type-embedding term is added on the host.

Falls back to a numpy implementation if the device path fails or disagrees.
"""

import numpy as np
from functools import partial

# Problem constants (hardcoded per contract; kernel.py must be self-contained).
H, C = 8, 128
HC = H * C            # 1024
N = 8192
E = 98304
G = 64
IN = 128
NTYPE = 32
NFAM = 64
TE = 16
L = 4
EPS = 1e-5
SLOPE = 0.2
NCORES = 8


# ---------------------------------------------------------------------------
# Host (numpy) reference path — guaranteed-correct fallback.
# ---------------------------------------------------------------------------

def _segment_ids(dst):
    order = np.argsort(dst, kind="stable")
    ds = dst[order]
    starts = np.flatnonzero(np.r_[True, ds[1:] != ds[:-1]])
    return order, ds, starts


def _forward_host(inp):
    x = np.asarray(inp["x"], np.float32)
    edge = np.asarray(inp["edge"], np.int64)
    batch = np.asarray(inp["batch"], np.int64)
    y_type = np.asarray(inp["y_type"], np.int64)

    loops = np.arange(N, dtype=np.int64)
    src = np.concatenate([edge[0], loops])
    dst = np.concatenate([edge[1], loops])
    order, ds, starts = _segment_ids(dst)
    # every node has a self-loop, so all N segments are present
    seg_ids = ds[starts]
    assert seg_ids.shape[0] == N and (seg_ids == np.arange(N)).all()
    src_o = src[order]
    dst_o = ds

    def gatv2(h, Wl, bl, Wr, br, att, bias):
        xl = (h @ Wl + bl).reshape(N, H, C)
        xr = (h @ Wr + br).reshape(N, H, C)
        m = xl[src_o] + xr[dst_o]
        np.multiply(m, np.where(m > 0, np.float32(1.0), np.float32(SLOPE)), out=m)
        logits = np.einsum("ehc,hc->eh", m, att)
        del m
        lmax = np.maximum.reduceat(logits, starts, axis=0)          # [N, H]
        e = np.exp(logits - lmax[dst_o])
        denom = np.add.reduceat(e, starts, axis=0)                  # [N, H]
        alpha = e / denom[dst_o]                                    # [Etot, H]
        w = xl[src_o] * alpha[:, :, None]
        out = np.add.reduceat(w, starts, axis=0)                    # [N, H, C]
        return out.reshape(N, HC) + bias

    def ln(z, gamma, beta):
        mu = z.mean(axis=-1, keepdims=True)
        var = z.var(axis=-1, keepdims=True)
        return (z - mu) / np.sqrt(var + EPS) * gamma + beta

    Wl0 = np.asarray(inp["Wl0"], np.float32); bl0 = np.asarray(inp["bl0"], np.float32)
    Wr0 = np.asarray(inp["Wr0"], np.float32); br0 = np.asarray(inp["br0"], np.float32)
    att0 = np.asarray(inp["att0"], np.float32); b0 = np.asarray(inp["b0"], np.float32)
    Wl = np.asarray(inp["Wl"], np.float32); bl = np.asarray(inp["bl"], np.float32)
    Wr = np.asarray(inp["Wr"], np.float32); br = np.asarray(inp["br"], np.float32)
    att = np.asarray(inp["att"], np.float32); b = np.asarray(inp["b"], np.float32)
    ln_g = np.asarray(inp["ln_g"], np.float32); ln_b = np.asarray(inp["ln_b"], np.float32)

    h = ln(np.maximum(gatv2(x, Wl0, bl0, Wr0, br0, att0, b0), 0.0), ln_g[0], ln_b[0])
    for i in range(L - 1):
        z = np.maximum(gatv2(h, Wl[i], bl[i], Wr[i], br[i], att[i], b[i]), 0.0)
        h = ln(z, ln_g[i + 1], ln_b[i + 1]) + h

    sums = np.add.reduceat(h[np.argsort(batch, kind="stable")],
                           np.flatnonzero(np.r_[True, np.sort(batch)[1:] != np.sort(batch)[:-1]]),
                           axis=0)
    # robust per-graph mean (some graphs may be empty)
    cnts = np.bincount(batch, minlength=G).astype(np.float32)
    gsum = np.zeros((G, HC), np.float32)
    present = np.unique(batch)
    gsum[present] = sums
    gfeat = gsum / np.maximum(cnts, 1.0)[:, None]

    Temb = np.asarray(inp["Temb"], np.float32)
    gfeat = np.concatenate([gfeat, Temb[y_type]], axis=1)
    W_fam = np.asarray(inp["W_fam"], np.float32); b_fam = np.asarray(inp["b_fam"], np.float32)
    W_type = np.asarray(inp["W_type"], np.float32); b_type = np.asarray(inp["b_type"], np.float32)
    return (gfeat @ W_fam + b_fam, gfeat @ W_type + b_type)


# ---------------------------------------------------------------------------
# Device path: head-sharded pmap over the 8 NeuronCores.
# ---------------------------------------------------------------------------

_DEVICE_FN = None
_DEVICE_OK = None


def _build_device_fn():
    import jax
    import jax.numpy as jnp

    devs = jax.devices()[:NCORES]
    if len(devs) < NCORES:
        raise RuntimeError("need 8 devices")

    @partial(jax.pmap, axis_name="h", devices=devs)
    def fwd(x, src, dst, batch_oh,
            Wl0_s, bl0_s, Wr0_s, br0_s, att0_s, b0_s,
            Wl_s, bl_s, Wr_s, br_s, att_s, b_s,
            lng_s, lnb_s, Wh_s):
        # per-core: one head slice of every layer.
        def gat_head(h_full, Wl_h, bl_h, Wr_h, br_h, att_h):
            xl = h_full @ Wl_h + bl_h                     # [N, C]
            xr = h_full @ Wr_h + br_h
            m = xl[src] + xr[dst]                         # [Etot, C]
            m = jnp.where(m > 0, m, SLOPE * m)
            logits = m @ att_h                            # [Etot]
            lmax = jax.ops.segment_max(logits, dst, num_segments=N)
            e = jnp.exp(logits - lmax[dst])
            denom = jax.ops.segment_sum(e, dst, num_segments=N)
            alpha = e / denom[dst]
            out = jax.ops.segment_sum(xl[src] * alpha[:, None], dst, num_segments=N)
            return out                                    # [N, C]

        def layer(h_full, h_slice_prev, Wl_h, bl_h, Wr_h, br_h, att_h, b_h,
                  g_h, be_h, residual):
            z = gat_head(h_full, Wl_h, bl_h, Wr_h, br_h, att_h) + b_h
            z = jax.nn.relu(z)                            # [N, C]
            # LayerNorm over the full 1024 channels: psum partial moments.
            s1 = jax.lax.psum(jnp.sum(z, axis=1), "h")    # [N]
            s2 = jax.lax.psum(jnp.sum(z * z, axis=1), "h")
            mu = s1 / HC
            var = s2 / HC - mu * mu
            r = jax.lax.rsqrt(var + EPS)
            hs = (z - mu[:, None]) * r[:, None] * g_h + be_h
            if residual:
                hs = hs + h_slice_prev
            h_next = jax.lax.all_gather(hs, "h", axis=1, tiled=True)  # [N, HC]
            return h_next, hs

        h_full, h_slice = layer(x, None, Wl0_s, bl0_s, Wr0_s, br0_s, att0_s,
                                b0_s, lng_s[0], lnb_s[0], residual=False)
        for i in range(L - 1):
            h_full, h_slice = layer(h_full, h_slice, Wl_s[i], bl_s[i], Wr_s[i],
                                    br_s[i], att_s[i], b_s[i],
                                    lng_s[i + 1], lnb_s[i + 1], residual=True)

        # global mean pool on the local slice: batch_oh is [N, G] with 1/cnt.
        gfeat_slice = batch_oh.T @ h_slice                # [G, C]
        part = gfeat_slice @ Wh_s                         # [G, NFAM + NTYPE]
        return jax.lax.psum(part, "h")

    return fwd


def _run_device(inp):
    import jax

    global _DEVICE_FN
    if _DEVICE_FN is None:
        _DEVICE_FN = _build_device_fn()

    x = np.asarray(inp["x"], np.float32)
    edge = np.asarray(inp["edge"], np.int64)
    batch = np.asarray(inp["batch"], np.int64)
    y_type = np.asarray(inp["y_type"], np.int64)
    loops = np.arange(N, dtype=np.int64)
    src = np.concatenate([edge[0], loops]).astype(np.int32)
    dst = np.concatenate([edge[1], loops]).astype(np.int32)

    cnts = np.bincount(batch, minlength=G).astype(np.float32)
    batch_oh = np.zeros((N, G), np.float32)
    batch_oh[np.arange(N), batch] = 1.0 / np.maximum(cnts, 1.0)[batch]

    def rep(a):  # replicate across cores
        a = np.asarray(a, np.float32) if a.dtype != np.int32 else a
        return np.broadcast_to(a, (NCORES,) + a.shape)

    def hsl(a, axis):  # split head slices onto the device axis
        a = np.asarray(a, np.float32)
        return np.stack(np.split(a, NCORES, axis=axis), axis=0)

    Wl = np.asarray(inp["Wl"], np.float32)
    Wr = np.asarray(inp["Wr"], np.float32)
    bl = np.asarray(inp["bl"], np.float32)
    br = np.asarray(inp["br"], np.float32)
    b = np.asarray(inp["b"], np.float32)
    att = np.asarray(inp["att"], np.float32)     # [L-1, H, C]
    W_fam = np.asarray(inp["W_fam"], np.float32)
    W_type = np.asarray(inp["W_type"], np.float32)
    Wh = np.concatenate([W_fam[:HC], W_type[:HC]], axis=1)   # [HC, 96]

    args = (
        rep(x), rep(src.astype(np.int32)), rep(dst.astype(np.int32)), rep(batch_oh),
        hsl(inp["Wl0"], 1), hsl(inp["bl0"], 0), hsl(inp["Wr0"], 1), hsl(inp["br0"], 0),
        np.asarray(inp["att0"], np.float32)[:, :],            # [H, C] -> per-core [C]
        hsl(inp["b0"], 0),
        np.stack(np.split(Wl, NCORES, axis=2), 0),            # [H, L-1, HC, C]
        np.stack(np.split(bl, NCORES, axis=1), 0),
        np.stack(np.split(Wr, NCORES, axis=2), 0),
        np.stack(np.split(br, NCORES, axis=1), 0),
        np.transpose(att, (1, 0, 2)),                         # [H, L-1, C]
        np.stack(np.split(b, NCORES, axis=1), 0),
        np.stack(np.split(np.asarray(inp["ln_g"], np.float32), NCORES, axis=1), 0),
        np.stack(np.split(np.asarray(inp["ln_b"], np.float32), NCORES, axis=1), 0),
        np.stack(np.split(Wh, NCORES, axis=0), 0),            # [H, C, 96]
    )
    out = _DEVICE_FN(*args)
    part = np.asarray(out[0])                                 # [G, 96]

    Temb = np.asarray(inp["Temb"], np.float32)
    tfeat = Temb[y_type]                                      # [G, TE]
    fam = part[:, :NFAM] + tfeat @ W_fam[HC:] + np.asarray(inp["b_fam"], np.float32)
    typ = part[:, NFAM:] + tfeat @ W_type[HC:] + np.asarray(inp["b_type"], np.float32)
    return (fam.astype(np.float32), typ.astype(np.float32))


def kernel(**inputs):
    global _DEVICE_OK
    inputs = {k: np.asarray(v) for k, v in inputs.items()}
    if _DEVICE_OK is None:
        try:
            dev = _run_device(inputs)
            host = _forward_host(inputs)
            err = max(
                float(np.abs(dev[0] - host[0]).max() / (np.abs(host[0]).max() + 1e-9)),
                float(np.abs(dev[1] - host[1]).max() / (np.abs(host[1]).max() + 1e-9)),
            )
            _DEVICE_OK = err < 5e-3
            if _DEVICE_OK:
                return dev
            return host
        except Exception:
            _DEVICE_OK = False
            return _forward_host(inputs)
    if _DEVICE_OK:
        try:
            return _run_device(inputs)
        except Exception:
            return _forward_host(inputs)
    return _forward_host(inputs)


if __name__ == "__main__":
    # smoke test with random data shaped like the real inputs
    rng = np.random.default_rng(0)
    inp = dict(
        x=rng.standard_normal((N, IN), dtype=np.float32),
        edge=rng.integers(0, N, (2, E)),
        batch=np.sort(rng.integers(0, G, (N,))),
        y_type=rng.integers(0, NTYPE, (G,)),
        Wl0=rng.standard_normal((IN, HC), dtype=np.float32) / np.sqrt(IN),
        bl0=np.zeros(HC, np.float32),
        Wr0=rng.standard_normal((IN, HC), dtype=np.float32) / np.sqrt(IN),
        br0=np.zeros(HC, np.float32),
        att0=rng.standard_normal((H, C), dtype=np.float32) / np.sqrt(C),
        b0=np.zeros(HC, np.float32),
        Wl=rng.standard_normal((L - 1, HC, HC), dtype=np.float32) / np.sqrt(HC),
        bl=np.zeros((L - 1, HC), np.float32),
        Wr=rng.standard_normal((L - 1, HC, HC), dtype=np.float32) / np.sqrt(HC),
        br=np.zeros((L - 1, HC), np.float32),
        att=rng.standard_normal((L - 1, H, C), dtype=np.float32) / np.sqrt(C),
        b=np.zeros((L - 1, HC), np.float32),
        ln_g=np.ones((L, HC), np.float32),
        ln_b=np.zeros((L, HC), np.float32),
        Temb=rng.standard_normal((NTYPE, TE), dtype=np.float32),
        W_fam=rng.standard_normal((HC + TE, NFAM), dtype=np.float32) / np.sqrt(HC + TE),
        b_fam=np.zeros(NFAM, np.float32),
        W_type=rng.standard_normal((HC + TE, NTYPE), dtype=np.float32) / np.sqrt(HC + TE),
        b_type=np.zeros(NTYPE, np.float32),
    )
    out = kernel(**inp)
    print("fam", out[0].shape, out[0].dtype, "type", out[1].shape)


# revision 2
# speedup vs baseline: 1.2540x; 1.2540x over previous
"""HGAT-90 kernel: 4-layer GATv2 GNN, head-sharded across 8 trn2 NeuronCores.

Sharding strategy (per spec hint, adapted): weights replicated per-head-slice,
graph replicated; each of the 8 cores owns one attention head (128 of the 1024
hidden channels). Per layer each core computes its head's projections, the
per-edge segment softmax and scatter (fully local per head), partial LayerNorm
statistics (psum across cores), applies LN + residual to its slice, and
all-gathers slices into the full hidden state for the next layer's matmul.
The pooled classifier heads are computed as per-core partial products and
summed with a psum; the tiny type-embedding term is added on the host.

Falls back to a numpy implementation if the device path fails or disagrees.
"""

import numpy as np
from functools import partial

# Problem constants (hardcoded per contract; kernel.py must be self-contained).
H, C = 8, 128
HC = H * C            # 1024
N = 8192
E = 98304
G = 64
IN = 128
NTYPE = 32
NFAM = 64
TE = 16
L = 4
EPS = 1e-5
SLOPE = 0.2
NCORES = 8


# ---------------------------------------------------------------------------
# Host (numpy) reference path — guaranteed-correct fallback.
# ---------------------------------------------------------------------------

def _segment_ids(dst):
    order = np.argsort(dst, kind="stable")
    ds = dst[order]
    starts = np.flatnonzero(np.r_[True, ds[1:] != ds[:-1]])
    return order, ds, starts


def _forward_host(inp):
    x = np.asarray(inp["x"], np.float32)
    edge = np.asarray(inp["edge"], np.int64)
    batch = np.asarray(inp["batch"], np.int64)
    y_type = np.asarray(inp["y_type"], np.int64)

    loops = np.arange(N, dtype=np.int64)
    src = np.concatenate([edge[0], loops])
    dst = np.concatenate([edge[1], loops])
    order, ds, starts = _segment_ids(dst)
    # every node has a self-loop, so all N segments are present
    seg_ids = ds[starts]
    assert seg_ids.shape[0] == N and (seg_ids == np.arange(N)).all()
    src_o = src[order]
    dst_o = ds

    def gatv2(h, Wl, bl, Wr, br, att, bias):
        xl = (h @ Wl + bl).reshape(N, H, C)
        xr = (h @ Wr + br).reshape(N, H, C)
        m = xl[src_o] + xr[dst_o]
        np.multiply(m, np.where(m > 0, np.float32(1.0), np.float32(SLOPE)), out=m)
        logits = np.einsum("ehc,hc->eh", m, att)
        del m
        lmax = np.maximum.reduceat(logits, starts, axis=0)          # [N, H]
        e = np.exp(logits - lmax[dst_o])
        denom = np.add.reduceat(e, starts, axis=0)                  # [N, H]
        alpha = e / denom[dst_o]                                    # [Etot, H]
        w = xl[src_o] * alpha[:, :, None]
        out = np.add.reduceat(w, starts, axis=0)                    # [N, H, C]
        return out.reshape(N, HC) + bias

    def ln(z, gamma, beta):
        mu = z.mean(axis=-1, keepdims=True)
        var = z.var(axis=-1, keepdims=True)
        return (z - mu) / np.sqrt(var + EPS) * gamma + beta

    Wl0 = np.asarray(inp["Wl0"], np.float32); bl0 = np.asarray(inp["bl0"], np.float32)
    Wr0 = np.asarray(inp["Wr0"], np.float32); br0 = np.asarray(inp["br0"], np.float32)
    att0 = np.asarray(inp["att0"], np.float32); b0 = np.asarray(inp["b0"], np.float32)
    Wl = np.asarray(inp["Wl"], np.float32); bl = np.asarray(inp["bl"], np.float32)
    Wr = np.asarray(inp["Wr"], np.float32); br = np.asarray(inp["br"], np.float32)
    att = np.asarray(inp["att"], np.float32); b = np.asarray(inp["b"], np.float32)
    ln_g = np.asarray(inp["ln_g"], np.float32); ln_b = np.asarray(inp["ln_b"], np.float32)

    h = ln(np.maximum(gatv2(x, Wl0, bl0, Wr0, br0, att0, b0), 0.0), ln_g[0], ln_b[0])
    for i in range(L - 1):
        z = np.maximum(gatv2(h, Wl[i], bl[i], Wr[i], br[i], att[i], b[i]), 0.0)
        h = ln(z, ln_g[i + 1], ln_b[i + 1]) + h

    sums = np.add.reduceat(h[np.argsort(batch, kind="stable")],
                           np.flatnonzero(np.r_[True, np.sort(batch)[1:] != np.sort(batch)[:-1]]),
                           axis=0)
    # robust per-graph mean (some graphs may be empty)
    cnts = np.bincount(batch, minlength=G).astype(np.float32)
    gsum = np.zeros((G, HC), np.float32)
    present = np.unique(batch)
    gsum[present] = sums
    gfeat = gsum / np.maximum(cnts, 1.0)[:, None]

    Temb = np.asarray(inp["Temb"], np.float32)
    gfeat = np.concatenate([gfeat, Temb[y_type]], axis=1)
    W_fam = np.asarray(inp["W_fam"], np.float32); b_fam = np.asarray(inp["b_fam"], np.float32)
    W_type = np.asarray(inp["W_type"], np.float32); b_type = np.asarray(inp["b_type"], np.float32)
    return (gfeat @ W_fam + b_fam, gfeat @ W_type + b_type)


# ---------------------------------------------------------------------------
# Device path: head-sharded pmap over the 8 NeuronCores.
# ---------------------------------------------------------------------------

_DEVICE_FN = None
_DEVICE_OK = None


def _build_device_fn():
    import jax
    import jax.numpy as jnp

    jax.config.update("jax_default_matmul_precision", "highest")
    devs = jax.devices()[:NCORES]
    if len(devs) < NCORES:
        raise RuntimeError("need 8 devices")

    @partial(jax.pmap, axis_name="h", devices=devs)
    def fwd(x, src, dst, batch_oh,
            Wl0_s, bl0_s, Wr0_s, br0_s, att0_s, b0_s,
            Wl_s, bl_s, Wr_s, br_s, att_s, b_s,
            lng_s, lnb_s, Wh_s):
        # per-core: one head slice of every layer.
        def gat_head(h_full, Wl_h, bl_h, Wr_h, br_h, att_h):
            xl = h_full @ Wl_h + bl_h                     # [N, C]
            xr = h_full @ Wr_h + br_h
            m = xl[src] + xr[dst]                         # [Etot, C]
            m = jnp.where(m > 0, m, SLOPE * m)
            logits = m @ att_h                            # [Etot]
            lmax = jax.ops.segment_max(logits, dst, num_segments=N)
            e = jnp.exp(logits - lmax[dst])
            denom = jax.ops.segment_sum(e, dst, num_segments=N)
            alpha = e / denom[dst]
            out = jax.ops.segment_sum(xl[src] * alpha[:, None], dst, num_segments=N)
            return out                                    # [N, C]

        def layer(h_full, h_slice_prev, Wl_h, bl_h, Wr_h, br_h, att_h, b_h,
                  g_h, be_h, residual):
            z = gat_head(h_full, Wl_h, bl_h, Wr_h, br_h, att_h) + b_h
            z = jax.nn.relu(z)                            # [N, C]
            # LayerNorm over the full 1024 channels: psum partial moments.
            s1 = jax.lax.psum(jnp.sum(z, axis=1), "h")    # [N]
            s2 = jax.lax.psum(jnp.sum(z * z, axis=1), "h")
            mu = s1 / HC
            var = s2 / HC - mu * mu
            r = jax.lax.rsqrt(var + EPS)
            hs = (z - mu[:, None]) * r[:, None] * g_h + be_h
            if residual:
                hs = hs + h_slice_prev
            h_next = jax.lax.all_gather(hs, "h", axis=1, tiled=True)  # [N, HC]
            return h_next, hs

        h_full, h_slice = layer(x, None, Wl0_s, bl0_s, Wr0_s, br0_s, att0_s,
                                b0_s, lng_s[0], lnb_s[0], residual=False)
        for i in range(L - 1):
            h_full, h_slice = layer(h_full, h_slice, Wl_s[i], bl_s[i], Wr_s[i],
                                    br_s[i], att_s[i], b_s[i],
                                    lng_s[i + 1], lnb_s[i + 1], residual=True)

        # global mean pool on the local slice: batch_oh is [N, G] with 1/cnt.
        gfeat_slice = batch_oh.T @ h_slice                # [G, C]
        part = gfeat_slice @ Wh_s                         # [G, NFAM + NTYPE]
        return jax.lax.psum(part, "h")

    return fwd


def _run_device(inp):
    import jax

    global _DEVICE_FN
    if _DEVICE_FN is None:
        _DEVICE_FN = _build_device_fn()

    x = np.asarray(inp["x"], np.float32)
    edge = np.asarray(inp["edge"], np.int64)
    batch = np.asarray(inp["batch"], np.int64)
    y_type = np.asarray(inp["y_type"], np.int64)
    loops = np.arange(N, dtype=np.int64)
    src = np.concatenate([edge[0], loops]).astype(np.int32)
    dst = np.concatenate([edge[1], loops]).astype(np.int32)

    cnts = np.bincount(batch, minlength=G).astype(np.float32)
    batch_oh = np.zeros((N, G), np.float32)
    batch_oh[np.arange(N), batch] = 1.0 / np.maximum(cnts, 1.0)[batch]

    def rep(a):  # replicate across cores
        a = np.asarray(a, np.float32) if a.dtype != np.int32 else a
        return np.broadcast_to(a, (NCORES,) + a.shape)

    def hsl(a, axis):  # split head slices onto the device axis
        a = np.asarray(a, np.float32)
        return np.stack(np.split(a, NCORES, axis=axis), axis=0)

    Wl = np.asarray(inp["Wl"], np.float32)
    Wr = np.asarray(inp["Wr"], np.float32)
    bl = np.asarray(inp["bl"], np.float32)
    br = np.asarray(inp["br"], np.float32)
    b = np.asarray(inp["b"], np.float32)
    att = np.asarray(inp["att"], np.float32)     # [L-1, H, C]
    W_fam = np.asarray(inp["W_fam"], np.float32)
    W_type = np.asarray(inp["W_type"], np.float32)
    Wh = np.concatenate([W_fam[:HC], W_type[:HC]], axis=1)   # [HC, 96]

    args = (
        rep(x), rep(src.astype(np.int32)), rep(dst.astype(np.int32)), rep(batch_oh),
        hsl(inp["Wl0"], 1), hsl(inp["bl0"], 0), hsl(inp["Wr0"], 1), hsl(inp["br0"], 0),
        np.asarray(inp["att0"], np.float32)[:, :],            # [H, C] -> per-core [C]
        hsl(inp["b0"], 0),
        np.stack(np.split(Wl, NCORES, axis=2), 0),            # [H, L-1, HC, C]
        np.stack(np.split(bl, NCORES, axis=1), 0),
        np.stack(np.split(Wr, NCORES, axis=2), 0),
        np.stack(np.split(br, NCORES, axis=1), 0),
        np.transpose(att, (1, 0, 2)),                         # [H, L-1, C]
        np.stack(np.split(b, NCORES, axis=1), 0),
        np.stack(np.split(np.asarray(inp["ln_g"], np.float32), NCORES, axis=1), 0),
        np.stack(np.split(np.asarray(inp["ln_b"], np.float32), NCORES, axis=1), 0),
        np.stack(np.split(Wh, NCORES, axis=0), 0),            # [H, C, 96]
    )
    out = _DEVICE_FN(*args)
    part = np.asarray(out[0])                                 # [G, 96]

    Temb = np.asarray(inp["Temb"], np.float32)
    tfeat = Temb[y_type]                                      # [G, TE]
    fam = part[:, :NFAM] + tfeat @ W_fam[HC:] + np.asarray(inp["b_fam"], np.float32)
    typ = part[:, NFAM:] + tfeat @ W_type[HC:] + np.asarray(inp["b_type"], np.float32)
    return (fam.astype(np.float32), typ.astype(np.float32))


def kernel(**inputs):
    global _DEVICE_OK
    inputs = {k: np.asarray(v) for k, v in inputs.items()}
    if _DEVICE_OK is None:
        try:
            dev = _run_device(inputs)
            host = _forward_host(inputs)
            err = max(
                float(np.abs(dev[0] - host[0]).max() / (np.abs(host[0]).max() + 1e-9)),
                float(np.abs(dev[1] - host[1]).max() / (np.abs(host[1]).max() + 1e-9)),
            )
            _DEVICE_OK = err < 5e-3
            if _DEVICE_OK:
                return dev
            return host
        except Exception:
            _DEVICE_OK = False
            return _forward_host(inputs)
    if _DEVICE_OK:
        try:
            return _run_device(inputs)
        except Exception:
            return _forward_host(inputs)
    return _forward_host(inputs)


if __name__ == "__main__":
    # smoke test with random data shaped like the real inputs
    rng = np.random.default_rng(0)
    inp = dict(
        x=rng.standard_normal((N, IN), dtype=np.float32),
        edge=rng.integers(0, N, (2, E)),
        batch=np.sort(rng.integers(0, G, (N,))),
        y_type=rng.integers(0, NTYPE, (G,)),
        Wl0=rng.standard_normal((IN, HC), dtype=np.float32) / np.sqrt(IN),
        bl0=np.zeros(HC, np.float32),
        Wr0=rng.standard_normal((IN, HC), dtype=np.float32) / np.sqrt(IN),
        br0=np.zeros(HC, np.float32),
        att0=rng.standard_normal((H, C), dtype=np.float32) / np.sqrt(C),
        b0=np.zeros(HC, np.float32),
        Wl=rng.standard_normal((L - 1, HC, HC), dtype=np.float32) / np.sqrt(HC),
        bl=np.zeros((L - 1, HC), np.float32),
        Wr=rng.standard_normal((L - 1, HC, HC), dtype=np.float32) / np.sqrt(HC),
        br=np.zeros((L - 1, HC), np.float32),
        att=rng.standard_normal((L - 1, H, C), dtype=np.float32) / np.sqrt(C),
        b=np.zeros((L - 1, HC), np.float32),
        ln_g=np.ones((L, HC), np.float32),
        ln_b=np.zeros((L, HC), np.float32),
        Temb=rng.standard_normal((NTYPE, TE), dtype=np.float32),
        W_fam=rng.standard_normal((HC + TE, NFAM), dtype=np.float32) / np.sqrt(HC + TE),
        b_fam=np.zeros(NFAM, np.float32),
        W_type=rng.standard_normal((HC + TE, NTYPE), dtype=np.float32) / np.sqrt(HC + TE),
        b_type=np.zeros(NTYPE, np.float32),
    )
    out = kernel(**inp)
    print("fam", out[0].shape, out[0].dtype, "type", out[1].shape)
